# revision 3
# baseline (speedup 1.0000x reference)
"""Soft-kNN imputation kernel for Trainium2 (8 NeuronCores, SPMD).

Problem: for a single query X_missing [64], over X_train [1M, 64]:
  d_i   = ||x_i - q||_2
  w_i   = softmax(-d_i)            (tau = 1.0)
  out   = sum over top-32 w_i * y_train[i]     -> [1, 64]

Sharding: X_train is split along N across the 8 cores (125,000 rows each,
padded to 125,056 = 128 partitions x 977 rows with sentinel rows whose
distance is huge -> exp underflows to exactly 0). Each core streams its
32 MB shard once (memory-bound), computing per-row squared distances with
DVE subtract + ACT square + DVE group-reduce, then sqrt / exp(-d) with an
accumulated per-partition partial softmax denominator, then an exact
per-partition top-32 via 4 rounds of the DVE max8 / max_index /
match_replace instructions. The host merges the 8 x 128 x 32 candidates
(any global top-32 element is necessarily in its own partition's top-32),
finishes the softmax normalization, and does the 32-row gather from
y_train plus the tiny weighted sum. y_train never touches the device -
only 32 of its rows are ever needed.
"""

import numpy as np

N = 1_000_000
D = 64
K = 32
NCORES = 8
SHARD = N // NCORES            # 125000 rows per core
PROWS = 128                    # SBUF partitions
RPP = 977                      # rows per partition (128*977 = 125056)
PAD_ROWS = PROWS * RPP         # padded shard rows
CHUNK = 64                     # rows per partition per supertile
PAD_VAL = 1.0e4                # sentinel: d ~ 8e4 -> exp(-d) == 0.0 in f32

_CACHE = {}
LAST_RESULTS = None            # BassKernelResults of the most recent run


def _build_nc():
    import concourse.bacc as bacc
    import concourse.tile as tile
    from concourse import mybir

    f32 = mybir.dt.float32

    # Bacc (not plain Bass): its compile() pipeline runs
    # generate_event_semaphores, which splits multi-semaphore waits into
    # event-semaphore chains — the TRN2 ISA allows at most one wait per
    # instruction and walrus rejects unsplit programs.
    nc = bacc.Bacc("TRN2", target_bir_lowering=False, debug=False)
    x_d = nc.dram_tensor("x", [PAD_ROWS, D], f32, kind="ExternalInput").ap()
    qb_d = nc.dram_tensor("qb", [PROWS, D], f32, kind="ExternalInput").ap()
    vals_d = nc.dram_tensor("cand_vals", [PROWS, K], f32, kind="ExternalOutput").ap()
    idx_d = nc.dram_tensor(
        "cand_idx", [PROWS, K], mybir.dt.uint32, kind="ExternalOutput"
    ).ap()
    z_d = nc.dram_tensor("z_part", [PROWS, 1], f32, kind="ExternalOutput").ap()

    # Partition p owns rows [p*RPP, (p+1)*RPP); within a partition the rows
    # are streamed CHUNK at a time, 16 KB contiguous DRAM per partition per
    # supertile DMA.
    xv = x_d.rearrange("(p r) d -> p (r d)", p=PROWS)

    with tile.TileContext(nc) as tc:
        with (
            tc.tile_pool(name="persist", bufs=1) as persist,
            tc.tile_pool(name="xs", bufs=4) as xs_pool,
        ):
            qb = persist.tile([PROWS, D], f32)
            nc.gpsimd.dma_start(out=qb[:], in_=qb_d[:])
            qb3 = qb.rearrange("p (o d) -> p o d", o=1)

            d2 = persist.tile([PROWS, RPP], f32)
            wt = persist.tile([PROWS, RPP], f32)
            vals = persist.tile([PROWS, K], f32)
            idxs = persist.tile([PROWS, K], mybir.dt.uint32)
            zp = persist.tile([PROWS, 1], f32)

            col = 0
            while col < RPP:
                r = min(CHUNK, RPP - col)
                fd = r * D
                xs = xs_pool.tile([PROWS, CHUNK * D], f32, tag="xs")
                nc.gpsimd.dma_start(
                    out=xs[:, :fd], in_=xv[:, col * D : col * D + fd]
                )
                x3 = xs[:, :fd].rearrange("p (r d) -> p r d", d=D)
                nc.vector.tensor_sub(x3, x3, qb3.to_broadcast([PROWS, r, D]))
                nc.scalar.activation(
                    xs[:, :fd], xs[:, :fd], mybir.ActivationFunctionType.Square
                )
                nc.vector.tensor_reduce(
                    out=d2[:, col : col + r],
                    in_=x3,
                    axis=mybir.AxisListType.X,
                    op=mybir.AluOpType.add,
                )
                col += r

            # d2 -> d -> w = exp(-d); zp[p] = sum_j w[p, j]
            nc.scalar.activation(d2[:], d2[:], mybir.ActivationFunctionType.Sqrt)
            nc.scalar.activation(
                wt[:],
                d2[:],
                mybir.ActivationFunctionType.Exp,
                scale=-1.0,
                accum_out=zp[:],
            )

            # Exact per-partition top-32 (descending) with column indices.
            for rnd in range(K // 8):
                v8 = vals[:, rnd * 8 : (rnd + 1) * 8]
                i8 = idxs[:, rnd * 8 : (rnd + 1) * 8]
                nc.vector.max(out=v8, in_=wt[:])
                nc.vector.max_index(out=i8, in_max=v8, in_values=wt[:])
                if rnd < K // 8 - 1:
                    nc.vector.match_replace(
                        out=wt[:], in_to_replace=v8, in_values=wt[:], imm_value=0.0
                    )

            nc.gpsimd.dma_start(out=vals_d[:], in_=vals[:])
            nc.gpsimd.dma_start(out=idx_d[:], in_=idxs[:])
            nc.gpsimd.dma_start(out=z_d[:], in_=zp[:])

    nc.compile()
    return nc


def kernel(X_train, y_train, X_missing):
    import os

    from concourse.bass_utils import run_bass_kernel_spmd

    global LAST_RESULTS

    X_train = np.ascontiguousarray(np.asarray(X_train, dtype=np.float32))
    y_train = np.asarray(y_train, dtype=np.float32)
    X_missing = np.asarray(X_missing, dtype=np.float32)

    if "nc" not in _CACHE:
        _CACHE["nc"] = _build_nc()
    nc = _CACHE["nc"]

    qb = np.ascontiguousarray(np.tile(X_missing[None, :], (PROWS, 1)))
    in_maps = []
    for c in range(NCORES):
        xp = np.full((PAD_ROWS, D), PAD_VAL, dtype=np.float32)
        xp[:SHARD] = X_train[c * SHARD : (c + 1) * SHARD]
        in_maps.append({"x": xp, "qb": qb})

    trace = bool(int(os.environ.get("KNN_TRACE", "0")))
    res = run_bass_kernel_spmd(
        nc, in_maps, core_ids=list(range(NCORES)), trace=trace
    )
    LAST_RESULTS = res

    # Host-side merge: global softmax denominator + global top-32 among the
    # per-partition top-32 candidates, then the 32-row gather from y_train.
    z_total = 0.0
    all_vals = []
    all_rows = []
    for c in range(NCORES):
        out_c = res.results[c]
        z_total += float(out_c["z_part"].astype(np.float64).sum())
        v = out_c["cand_vals"].reshape(-1)
        j = out_c["cand_idx"].astype(np.int64).reshape(PROWS, K)
        local_row = np.arange(PROWS, dtype=np.int64)[:, None] * RPP + j
        rows = c * SHARD + local_row.reshape(-1)
        keep = (local_row.reshape(-1) < SHARD) & (v > 0)
        all_vals.append(v[keep])
        all_rows.append(rows[keep])
    all_vals = np.concatenate(all_vals)
    all_rows = np.concatenate(all_rows)

    sel = np.argpartition(-all_vals, K - 1)[:K]
    w = all_vals[sel].astype(np.float64) / z_total
    out = (w[:, None] * y_train[all_rows[sel]].astype(np.float64)).sum(axis=0)
    return out[None, :].astype(np.float32)
